# revision 24
# baseline (speedup 1.0000x reference)
"""Self-contained Trainium2 Bass kernel for nn_MultiHeadAttention_68367289417808.

kernel(**inputs) takes FULL unsharded inputs (as in reference.setup_inputs())
and returns the FULL [4, 2048, 1024] output.

Sharding: 8 cores = (batch 4) x (query-half 2); no collectives needed.

Per-core pipeline (mixed precision, tuned against the TRN2 cost model):
  - Q/K projections in fp8e4m3 DoubleRow (weights x32, outputs stored as
    8*qh in fp8), V projection in fp8e4m3 DoubleRow with vh stored bf16.
  - scores = kh^T qh per head via fp8 DoubleRow (dk=64 split into two
    32-row k-tiles living on the same 32 partitions).
  - exp on the Act engine straight out of PSUM into bf16 (scale 2^-9
    compensates the 8x8 operand scaling); masked positions are then
    overwritten with exactly 1.0 (= exp(-1e-6) to ulp) via one DVE
    copy_predicated against a ones tile, matching the reference's
    masked_fill(-1e-6) semantics with no correction terms.
  - AV + softmax denominator (ones column in vaug) in bf16 matmuls,
    normalize on DVE (reciprocal + scalar_tensor_tensor), output
    projection + bias in bf16 with the final copy on the Act engine.
"""
import time

import jax
import ml_dtypes
import numpy as np
from jax.experimental.shard_map import shard_map
from jax.sharding import Mesh, PartitionSpec

import concourse.bass as bass
import concourse.bacc as bacc
import concourse.mybir as mybir
import concourse.tile as tile
from concourse import bass2jax
from concourse.bass import ts, ds

F32 = mybir.dt.float32
F8 = mybir.dt.float8e4
BF = mybir.dt.bfloat16
U16 = mybir.dt.uint16
FR = mybir.dt.float32r
AF = mybir.ActivationFunctionType
PM = mybir.MatmulPerfMode
MULT = mybir.AluOpType.mult
ADD = mybir.AluOpType.add

NF8 = ml_dtypes.float8_e4m3
NBF = ml_dtypes.bfloat16

P = 128
SQ = 1024
SK = 2048
D = 1024
H = 16
DK = 64
HK = 1024


def build_mha(phases=('proj', 'attn', 'oproj')):
    nc = bacc.Bacc("TRN2", target_bir_lowering=False)

    qT8 = nc.dram_tensor("qT8", [D, SQ], F8, kind="ExternalInput")
    kT8 = nc.dram_tensor("kT8", [D, SK], F8, kind="ExternalInput")
    vT16 = nc.dram_tensor("vT16", [D, SK], BF, kind="ExternalInput")
    mskT = nc.dram_tensor("mskT", [SK, SQ], U16, kind="ExternalInput")
    wq8 = nc.dram_tensor("wq8", [D, HK], F8, kind="ExternalInput")
    wk8 = nc.dram_tensor("wk8", [D, HK], F8, kind="ExternalInput")
    wv16 = nc.dram_tensor("wv16", [D, HK], BF, kind="ExternalInput")
    wo16 = nc.dram_tensor("wo16", [HK, D], BF, kind="ExternalInput")
    bqc = nc.dram_tensor("bqc", [P, 8], F32, kind="ExternalInput")
    bkc = nc.dram_tensor("bkc", [P, 8], F32, kind="ExternalInput")
    bvr = nc.dram_tensor("bvr", [1, HK], BF, kind="ExternalInput")
    bor = nc.dram_tensor("bor", [1, D], BF, kind="ExternalInput")
    out = nc.dram_tensor("out", [SQ, D], F32, kind="ExternalOutput")

    qhT_d = nc.dram_tensor("qhT_scr", [HK, SQ], F8)
    khT_d = nc.dram_tensor("khT_scr", [HK, SK], F8)
    rden_d = nc.dram_tensor("rden_scr", [H, SQ], F32)

    with tile.TileContext(nc) as tc:
        with tc.tile_pool(name="persist", bufs=1) as pers:
            msk_sb = pers.tile([P, 16, SQ], U16, tag="msk")
            vaug = pers.tile([P, 16, 65 * H], BF, tag="vaug")
            ao_sb = pers.tile([P, 8, SQ], BF, tag="ao")
            wo_sb = pers.tile([P, 8, D], BF, tag="wo")
            ones16 = pers.tile([P, SQ], BF, tag="ones16")
            bor_sb = pers.tile([1, D], BF, tag="bor")

            nc.vector.memset(ones16[:], 1.0)
            nc.gpsimd.memset(vaug[:].rearrange("p t c -> p (t c)"), 1.0)

            # ---------------- Q/K/V projections ----------------
            with (
                tc.tile_pool(name="wp", bufs=1) as wp,
                tc.tile_pool(name="xp", bufs=1) as xp,
                tc.tile_pool(name="pp", bufs=4, space="PSUM") as pp,
                tc.tile_pool(name="stg", bufs=3) as stg,
            ):
                wq_sb = wp.tile([P, 8, HK], F8, tag="wq")
                wk_sb = wp.tile([P, 8, HK], F8, tag="wk")
                wv_sb = wp.tile([P, 8, HK], BF, tag="wv")
                bvr_sb = wp.tile([1, HK], BF, tag="bvr")
                bq_sb = wp.tile([P, 8], F32, tag="bq")
                bk_sb = wp.tile([P, 8], F32, tag="bk")
                qx = xp.tile([P, 8, SQ], F8, tag="qx")
                kx = xp.tile([P, 8, SK], F8, tag="kx")
                vx = xp.tile([P, 8, SK], BF, tag="vx")
                for _j in range(8):
                    nc.sync.dma_start(
                        wq_sb[:, _j],
                        wq8.ap().rearrange("(j p) m -> p j m", p=P)[:, _j],
                    )
                    nc.sync.dma_start(
                        wk_sb[:, _j],
                        wk8.ap().rearrange("(j p) m -> p j m", p=P)[:, _j],
                    )
                    nc.sync.dma_start(
                        wv_sb[:, _j],
                        wv16.ap().rearrange("(j p) m -> p j m", p=P)[:, _j],
                    )
                    nc.sync.dma_start(
                        qx[:, _j],
                        qT8.ap().rearrange("(j p) s -> p j s", p=P)[:, _j],
                    )
                    nc.sync.dma_start(
                        kx[:, _j],
                        kT8.ap().rearrange("(j p) s -> p j s", p=P)[:, _j],
                    )
                    nc.sync.dma_start(
                        vx[:, _j],
                        vT16.ap().rearrange("(j p) s -> p j s", p=P)[:, _j],
                    )
                nc.sync.dma_start(bvr_sb[:], bvr.ap())
                nc.sync.dma_start(bq_sb[:], bqc.ap())
                nc.sync.dma_start(bk_sb[:], bkc.ap())

                # Q projection -> qhT_d (fp8, value 8*qh)
                for i in range(8 if ('proj' in phases or 'q' in phases) else 0):
                    for ct in range(SQ // 512):
                        ps = pp.tile([P, 512], F32, tag="pp", name=f"psq{i}_{ct}")
                        for cc in range(2):
                            for u in range(4):
                                nc.tensor.matmul(
                                    ps[:, ds(256 * cc, 256)],
                                    wq_sb[:, ds(2 * u, 2), ts(i, P)],
                                    qx[:, ds(2 * u, 2), ds(512 * ct + 256 * cc, 256)],
                                    start=(u == 0),
                                    stop=(u == 3),
                                    perf_mode=PM.DoubleRow,
                                )
                        st = stg.tile([P, 512], F8, tag="stg")
                        nc.vector.tensor_scalar(
                            st[:], ps[:], 0.25, bq_sb[:, i : i + 1], MULT, ADD
                        )
                        nc.sync.dma_start(qhT_d.ap()[ts(i, P), ts(ct, 512)], st[:])

                # K projection -> khT_d (fp8, value 8*kh)
                for i in range(8 if ('proj' in phases or 'k' in phases) else 0):
                    for ct in range(SK // 512):
                        ps = pp.tile([P, 512], F32, tag="pp", name=f"psk{i}_{ct}")
                        for cc in range(2):
                            for u in range(4):
                                nc.tensor.matmul(
                                    ps[:, ds(256 * cc, 256)],
                                    wk_sb[:, ds(2 * u, 2), ts(i, P)],
                                    kx[:, ds(2 * u, 2), ds(512 * ct + 256 * cc, 256)],
                                    start=(u == 0),
                                    stop=(u == 3),
                                    perf_mode=PM.DoubleRow,
                                )
                        st = stg.tile([P, 512], F8, tag="stg")
                        nc.vector.tensor_scalar(
                            st[:], ps[:], 0.25, bk_sb[:, i : i + 1], MULT, ADD
                        )
                        nc.sync.dma_start(khT_d.ap()[ts(i, P), ts(ct, 512)], st[:])

                # V projection (swapped operands, bf16): psum [sk-tile,
                # hk-chunk] -> copy into vaug (ones cols pre-set by memset)
                for t in range(16 if ('proj' in phases or 'v' in phases) else 0):
                    for c in range(2):
                        ps = pp.tile([P, 512], F32, tag="pp", name=f"psv{t}_{c}")
                        for u in range(8):
                            nc.tensor.matmul(
                                ps[:],
                                vx[:, u, ts(t, P)],
                                wv_sb[:, u, ds(512 * c, 512)],
                                start=(u == 0),
                                stop=False,
                            )
                        nc.tensor.matmul(
                            ps[:],
                            ones16[0:1, 0:P],
                            bvr_sb[:, ds(512 * c, 512)],
                            start=False,
                            stop=True,
                        )
                        nc.vector.tensor_copy(
                            vaug[:, t, ds(520 * c, 520)]
                            .rearrange("p (h x) -> p h x", x=65)[:, :, 0:64],
                            ps[:].rearrange("p (h x) -> p h x", x=64),
                        )

            # deferred big loads: mask (quartered) + Wo + bo ride the DMA
            # queues while the projections run
            for quar in range(4):
                nc.sync.dma_start(
                    msk_sb[:, ds(4 * quar, 4), :],
                    mskT.ap().rearrange("(t p) s -> p t s", p=P)[
                        :, ds(4 * quar, 4), :
                    ],
                )
            nc.sync.dma_start(bor_sb[:], bor.ap())
            for _j in range(8):
                nc.sync.dma_start(
                    wo_sb[:, _j],
                    wo16.ap().rearrange("(j p) m -> p j m", p=P)[:, _j],
                )

            # ---------------- attention ----------------
            if 'attn' not in phases:
                nc.compile()
                return nc
            with (
                tc.tile_pool(name="qkp", bufs=2) as qkp,
                tc.tile_pool(name="ep", bufs=4) as ep,
                tc.tile_pool(name="sps", bufs=2, space="PSUM") as sps,
                tc.tile_pool(name="avp", bufs=1, space="PSUM") as avp,
                tc.tile_pool(name="totp", bufs=2) as totp,
                tc.tile_pool(name="rbp", bufs=2) as rbp,
                tc.tile_pool(name="rbps", bufs=1, space="PSUM") as rbps,
            ):
                onesr = rbp.tile([1, 64], FR, tag="onesr")
                nc.vector.tensor_copy(onesr[:], ones16[0:1, 0:64])
                qk_tiles = {}
                def load_head(h):
                    qhs = qkp.tile([32, 2, SQ], F8, tag="qhs", name=f"qhs{h}")
                    khs = qkp.tile([32, 2, SK], F8, tag="khs", name=f"khs{h}")
                    nc.sync.dma_start(
                        qhs[:],
                        qhT_d.ap().rearrange(
                            "(h half p) s -> p h half s", half=2, p=32
                        )[:, h],
                    )
                    nc.sync.dma_start(
                        khs[:],
                        khT_d.ap().rearrange(
                            "(h half p) s -> p h half s", half=2, p=32
                        )[:, h],
                    )
                    qk_tiles[h] = (qhs, khs)

                load_head(0)
                load_head(1)
                pending = [None]
                for h in range(H):
                    qhs, khs = qk_tiles.pop(h)
                    if h + 2 < H:
                        load_head(h + 2)
                    pso = avp.tile([65, SQ], F32, tag="pso", name=f"pso{h}")
                    for t in range(16):
                        if t == 2 and pending[0] is not None:
                            pending[0]()
                            pending[0] = None
                        ps = sps.tile([P, SQ], F32, tag="sps", name=f"s{h}_{t}")
                        for c in range(4):
                            nc.tensor.matmul(
                                ps[:, ds(256 * c, 256)],
                                khs[:, :, ts(t, P)],
                                qhs[:, :, ds(256 * c, 256)],
                                start=True,
                                stop=True,
                                perf_mode=PM.DoubleRow,
                            )
                        e = ep.tile([P, SQ], BF, tag="e", name=f"e{h}_{t}")
                        nc.scalar.activation(e[:], ps[:], AF.Exp, scale=2.0 ** -9)
                        nc.vector.copy_predicated(e[:], msk_sb[:, t, :], ones16[:])
                        for c2 in range(2):
                            nc.tensor.matmul(
                                pso[:, ds(512 * c2, 512)],
                                vaug[:, t, ds(65 * h, 65)],
                                e[:, ds(512 * c2, 512)],
                                start=(t == 0),
                                stop=(t == 15),
                            )
                    def make_norm(h, pso):
                        def emit():
                            tot = totp.tile([65, SQ], F32, tag="tot", name=f"tot{h}")
                            nc.vector.tensor_copy(tot[:], pso[:])
                            rcp = rbp.tile([1, SQ], FR, tag="rcp", name=f"rcp{h}")
                            with nc.allow_low_precision(reason="f32r recip, multiplicative use"):
                                nc.vector.reciprocal(rcp[:], tot[64:65, :])
                            rb = rbps.tile([64, SQ], F32, tag="rb", name=f"rb{h}")
                            for _cb in range(2):
                                nc.tensor.matmul(
                                    rb[:, ds(512 * _cb, 512)],
                                    onesr[:, 0:64],
                                    rcp[:, ds(512 * _cb, 512)],
                                    start=True,
                                    stop=True,
                                )
                            nc.vector.scalar_tensor_tensor(
                                ao_sb[ds(64 * (h % 2), 64), h // 2, :],
                                tot[0:64, :],
                                1.0,
                                rb[:],
                                MULT,
                                MULT,
                            )
                        return emit
                    pending[0] = make_norm(h, pso)
                pending[0]()

            # ---------------- output projection ----------------
            if 'oproj' not in phases:
                nc.compile()
                return nc
            with (
                tc.tile_pool(name="pp2", bufs=4, space="PSUM") as pp2,
                tc.tile_pool(name="ost", bufs=3) as ost,
            ):
                for s in range(8):
                    for c in range(2):
                        ps = pp2.tile([P, 512], F32, tag="op", name=f"po{s}_{c}")
                        for g in range(8):
                            nc.tensor.matmul(
                                ps[:],
                                ao_sb[:, g, ts(s, P)],
                                wo_sb[:, g, ds(512 * c, 512)],
                                start=(g == 0),
                                stop=False,
                            )
                        nc.tensor.matmul(
                            ps[:],
                            ones16[0:1, 0:P],
                            bor_sb[:, ds(512 * c, 512)],
                            start=False,
                            stop=True,
                        )
                        oo = ost.tile([P, 512], F32, tag="oo")
                        nc.scalar.activation(oo[:], ps[:], AF.Copy, scale=1.0)
                        nc.sync.dma_start(out.ap()[ts(s, P), ds(512 * c, 512)], oo[:])

    nc.compile()
    return nc


def make_host_inputs(q, k, v, mask, Wq, bq, Wk, bk, Wv, bv, Wo, bo):
    """Full inputs -> list of 8 per-core input dicts."""
    q = np.asarray(q, np.float32)
    k = np.asarray(k, np.float32)
    v = np.asarray(v, np.float32)
    mask = np.asarray(mask)

    def f8(a):
        return np.ascontiguousarray(a, dtype=np.float32).astype(NF8)

    def bf(a):
        return np.ascontiguousarray(a, dtype=np.float32).astype(NBF)

    shared = {
        "wq8": f8(32.0 * np.asarray(Wq, np.float32).transpose(1, 0, 2).reshape(D, HK)),
        "wk8": f8(32.0 * np.asarray(Wk, np.float32).transpose(1, 0, 2).reshape(D, HK)),
        "wv16": bf(np.asarray(Wv, np.float32).transpose(1, 0, 2).reshape(D, HK)),
        "wo16": bf(np.asarray(Wo, np.float32)),
        "bqc": np.ascontiguousarray(
            8.0 * np.asarray(bq, np.float32).reshape(HK).reshape(8, P).T
        ),
        "bkc": np.ascontiguousarray(
            8.0 * np.asarray(bk, np.float32).reshape(HK).reshape(8, P).T
        ),
        "bvr": bf(np.asarray(bv, np.float32).reshape(1, HK)),
        "bor": bf(np.asarray(bo, np.float32).reshape(1, D)),
    }

    in_maps = []
    for core in range(8):
        b, j = divmod(core, 2)
        qs = q[b, j * SQ : (j + 1) * SQ, :]
        ms = mask[b, j * SQ : (j + 1) * SQ, :]
        m = dict(shared)
        m["qT8"] = f8(qs.T)
        m["kT8"] = f8(k[b].T)
        m["vT16"] = bf(v[b].T)
        m["mskT"] = np.ascontiguousarray((~ms).T).astype(np.uint16)
        in_maps.append(m)
    return in_maps


def assemble_output(results):
    """8 per-core out [SQ, D] -> full [4, 2048, 1024]."""
    B, S = 4, 2048
    full = np.empty((B, S, D), np.float32)
    for core, res in enumerate(results):
        b, j = divmod(core, 2)
        full[b, j * SQ : (j + 1) * SQ, :] = res["out"]
    return full


class CompiledSpmd:
    def __init__(self, nc: bass.Bass, n_cores: int):
        bass2jax.install_neuronx_cc_hook()
        assert nc.dbg_addr is None, "build with debug=False"
        partition_name = (
            nc.partition_id_tensor.name if nc.partition_id_tensor else None
        )
        in_names, out_names, out_avals, zero_outs = [], [], [], []
        for alloc in nc.m.functions[0].allocations:
            if not isinstance(alloc, mybir.MemoryLocationSet):
                continue
            name = alloc.memorylocations[0].name
            if alloc.kind == "ExternalInput":
                if name != partition_name:
                    in_names.append(name)
            elif alloc.kind == "ExternalOutput":
                shape = tuple(alloc.tensor_shape)
                dtype = mybir.dt.np(alloc.dtype)
                out_names.append(name)
                out_avals.append(jax.core.ShapedArray(shape, dtype))
                zero_outs.append(np.zeros(shape, dtype))
        n_params = len(in_names)
        n_outs = len(out_avals)
        all_in_names = list(in_names) + list(out_names)
        if partition_name is not None:
            all_in_names.append(partition_name)

        def _body(*args):
            operands = list(args)
            if partition_name is not None:
                operands.append(bass2jax.partition_id_tensor())
            outs = bass2jax._bass_exec_p.bind(
                *operands,
                out_avals=tuple(out_avals),
                in_names=tuple(all_in_names),
                out_names=tuple(out_names),
                lowering_input_output_aliases=(),
                sim_require_finite=True,
                sim_require_nnan=True,
                nc=nc,
            )
            return tuple(outs)

        devices = jax.devices()[:n_cores]
        assert len(devices) == n_cores
        mesh = Mesh(np.asarray(devices), ("core",))
        self._mesh = mesh
        donate = tuple(range(n_params, n_params + n_outs))
        self._sharded = jax.jit(
            shard_map(
                _body,
                mesh=mesh,
                in_specs=(PartitionSpec("core"),) * (n_params + n_outs),
                out_specs=(PartitionSpec("core"),) * n_outs,
                check_rep=False,
            ),
            donate_argnums=donate,
            keep_unused=True,
        )
        self.in_names = in_names
        self.out_names = out_names
        self.out_avals = out_avals
        self.zero_outs = zero_outs
        self.n_cores = n_cores

    def _concat_inputs(self, in_maps):
        per_core = [[np.asarray(m[n]) for n in self.in_names] for m in in_maps]
        return [
            np.concatenate([per_core[c][i] for c in range(self.n_cores)], axis=0)
            for i in range(len(self.in_names))
        ]

    def run(self, in_maps, repeats: int = 1):
        """Returns (results_per_core, wall_times_s list of len repeats)."""
        from jax.sharding import NamedSharding

        mesh = self._mesh
        shard = NamedSharding(mesh, PartitionSpec("core"))
        concat_in = [
            jax.device_put(a, shard) for a in self._concat_inputs(in_maps)
        ]
        rep_zeros = [
            [
                jax.device_put(
                    np.zeros((self.n_cores * z.shape[0], *z.shape[1:]), z.dtype),
                    shard,
                )
                for z in self.zero_outs
            ]
            for _ in range(repeats)
        ]
        jax.block_until_ready(concat_in)
        jax.block_until_ready(rep_zeros)
        times = []
        out_arrs = None
        for r in range(repeats):
            t0 = time.perf_counter()
            out_arrs = self._sharded(*concat_in, *rep_zeros[r])
            jax.block_until_ready(out_arrs)
            times.append(time.perf_counter() - t0)
        results = [
            {
                name: np.asarray(out_arrs[i]).reshape(
                    self.n_cores, *self.out_avals[i].shape
                )[c]
                for i, name in enumerate(self.out_names)
            }
            for c in range(self.n_cores)
        ]
        return results, times


_COMPILED = None


def _get_compiled():
    global _COMPILED
    if _COMPILED is None:
        nc = build_mha()
        _COMPILED = CompiledSpmd(nc, 8)
    return _COMPILED


def kernel(**inputs) -> np.ndarray:
    comp = _get_compiled()
    in_maps = make_host_inputs(**inputs)
    results, _ = comp.run(in_maps, repeats=1)
    return assemble_output(results)


# revision 26
# speedup vs baseline: 1.0352x; 1.0352x over previous
"""Self-contained Trainium2 Bass kernel for nn_MultiHeadAttention_68367289417808.

kernel(**inputs) takes FULL unsharded inputs (as in reference.setup_inputs())
and returns the FULL [4, 2048, 1024] output.

Sharding: 8 cores = (batch 4) x (query-half 2); no collectives needed.

Per-core pipeline (mixed precision, tuned against the TRN2 cost model):
  - Q/K projections in fp8e4m3 DoubleRow (weights x32, outputs stored as
    8*qh in fp8), V projection in fp8e4m3 DoubleRow with vh stored bf16.
  - scores = kh^T qh per head via fp8 DoubleRow (dk=64 split into two
    32-row k-tiles living on the same 32 partitions).
  - exp on the Act engine straight out of PSUM into bf16 (scale 2^-9
    compensates the 8x8 operand scaling); masked positions are then
    overwritten with exactly 1.0 (= exp(-1e-6) to ulp) via one DVE
    copy_predicated against a ones tile, matching the reference's
    masked_fill(-1e-6) semantics with no correction terms.
  - AV + softmax denominator (ones column in vaug) in bf16 matmuls,
    normalize on DVE (reciprocal + scalar_tensor_tensor), output
    projection + bias in bf16 with the final copy on the Act engine.
"""
import time

import jax
import ml_dtypes
import numpy as np
from jax.experimental.shard_map import shard_map
from jax.sharding import Mesh, PartitionSpec

import concourse.bass as bass
import concourse.bacc as bacc
import concourse.mybir as mybir
import concourse.tile as tile
from concourse import bass2jax
from concourse.bass import ts, ds

F32 = mybir.dt.float32
F8 = mybir.dt.float8e4
BF = mybir.dt.bfloat16
U16 = mybir.dt.uint16
FR = mybir.dt.float32r
AF = mybir.ActivationFunctionType
PM = mybir.MatmulPerfMode
MULT = mybir.AluOpType.mult
ADD = mybir.AluOpType.add

NF8 = ml_dtypes.float8_e4m3
NBF = ml_dtypes.bfloat16

P = 128
SQ = 1024
SK = 2048
D = 1024
H = 16
DK = 64
HK = 1024


def build_mha(phases=('proj', 'attn', 'oproj')):
    nc = bacc.Bacc("TRN2", target_bir_lowering=False)

    qT8 = nc.dram_tensor("qT8", [D, SQ], F8, kind="ExternalInput")
    kT8 = nc.dram_tensor("kT8", [D, SK], F8, kind="ExternalInput")
    vT16 = nc.dram_tensor("vT16", [D, SK], BF, kind="ExternalInput")
    mskT = nc.dram_tensor("mskT", [SK, SQ], U16, kind="ExternalInput")
    wq8 = nc.dram_tensor("wq8", [D, HK], F8, kind="ExternalInput")
    wk8 = nc.dram_tensor("wk8", [D, HK], F8, kind="ExternalInput")
    wv16 = nc.dram_tensor("wv16", [D, HK], BF, kind="ExternalInput")
    wo16 = nc.dram_tensor("wo16", [HK, D], BF, kind="ExternalInput")
    bqc = nc.dram_tensor("bqc", [P, 8], F32, kind="ExternalInput")
    bkc = nc.dram_tensor("bkc", [P, 8], F32, kind="ExternalInput")
    bvr = nc.dram_tensor("bvr", [1, HK], BF, kind="ExternalInput")
    bor = nc.dram_tensor("bor", [1, D], BF, kind="ExternalInput")
    out = nc.dram_tensor("out", [SQ, D], F32, kind="ExternalOutput")

    qhT_d = nc.dram_tensor("qhT_scr", [HK, SQ], F8)
    khT_d = nc.dram_tensor("khT_scr", [HK, SK], F8)
    rden_d = nc.dram_tensor("rden_scr", [H, SQ], F32)

    with tile.TileContext(nc) as tc:
        with tc.tile_pool(name="persist", bufs=1) as pers:
            msk_sb = pers.tile([P, 16, SQ], U16, tag="msk")
            vaug = pers.tile([P, 16, 65 * H], BF, tag="vaug")
            ao_sb = pers.tile([P, 8, SQ], BF, tag="ao")
            wo_sb = pers.tile([P, 8, D], BF, tag="wo")
            ones16 = pers.tile([P, SQ], BF, tag="ones16")
            bor_sb = pers.tile([1, D], BF, tag="bor")

            nc.vector.memset(ones16[:], 1.0)
            nc.gpsimd.memset(vaug[:].rearrange("p t c -> p (t c)"), 1.0)

            # ---------------- Q/K/V projections ----------------
            with (
                tc.tile_pool(name="wp", bufs=1) as wp,
                tc.tile_pool(name="xp", bufs=1) as xp,
                tc.tile_pool(name="pp", bufs=4, space="PSUM") as pp,
                tc.tile_pool(name="stg", bufs=3) as stg,
            ):
                wq_sb = wp.tile([P, 8, HK], F8, tag="wq")
                wk_sb = wp.tile([P, 8, HK], F8, tag="wk")
                wv_sb = wp.tile([P, 8, HK], BF, tag="wv")
                bvr_sb = wp.tile([1, HK], BF, tag="bvr")
                bq_sb = wp.tile([P, 8], F32, tag="bq")
                bk_sb = wp.tile([P, 8], F32, tag="bk")
                qx = xp.tile([P, 8, SQ], F8, tag="qx")
                kx = xp.tile([P, 8, SK], F8, tag="kx")
                vx = xp.tile([P, 8, SK], BF, tag="vx")
                for _j in range(8):
                    nc.sync.dma_start(
                        wq_sb[:, _j],
                        wq8.ap().rearrange("(j p) m -> p j m", p=P)[:, _j],
                    )
                    nc.sync.dma_start(
                        wk_sb[:, _j],
                        wk8.ap().rearrange("(j p) m -> p j m", p=P)[:, _j],
                    )
                    nc.sync.dma_start(
                        wv_sb[:, _j],
                        wv16.ap().rearrange("(j p) m -> p j m", p=P)[:, _j],
                    )
                    nc.sync.dma_start(
                        qx[:, _j],
                        qT8.ap().rearrange("(j p) s -> p j s", p=P)[:, _j],
                    )
                    nc.sync.dma_start(
                        kx[:, _j],
                        kT8.ap().rearrange("(j p) s -> p j s", p=P)[:, _j],
                    )
                    nc.sync.dma_start(
                        vx[:, _j],
                        vT16.ap().rearrange("(j p) s -> p j s", p=P)[:, _j],
                    )
                nc.sync.dma_start(bvr_sb[:], bvr.ap())
                nc.sync.dma_start(bq_sb[:], bqc.ap())
                nc.sync.dma_start(bk_sb[:], bkc.ap())

                # Q projection -> qhT_d (fp8, value 8*qh)
                for i in range(8 if ('proj' in phases or 'q' in phases) else 0):
                    for ct in range(SQ // 512):
                        ps = pp.tile([P, 512], F32, tag="pp", name=f"psq{i}_{ct}")
                        for cc in range(2):
                            for u in range(4):
                                nc.tensor.matmul(
                                    ps[:, ds(256 * cc, 256)],
                                    wq_sb[:, ds(2 * u, 2), ts(i, P)],
                                    qx[:, ds(2 * u, 2), ds(512 * ct + 256 * cc, 256)],
                                    start=(u == 0),
                                    stop=(u == 3),
                                    perf_mode=PM.DoubleRow,
                                )
                        st = stg.tile([P, 512], F8, tag="stg")
                        nc.vector.tensor_scalar(
                            st[:], ps[:], 0.25, bq_sb[:, i : i + 1], MULT, ADD
                        )
                        nc.sync.dma_start(qhT_d.ap()[ts(i, P), ts(ct, 512)], st[:])

                # K projection -> khT_d (fp8, value 8*kh)
                for i in range(8 if ('proj' in phases or 'k' in phases) else 0):
                    for ct in range(SK // 512):
                        ps = pp.tile([P, 512], F32, tag="pp", name=f"psk{i}_{ct}")
                        for cc in range(2):
                            for u in range(4):
                                nc.tensor.matmul(
                                    ps[:, ds(256 * cc, 256)],
                                    wk_sb[:, ds(2 * u, 2), ts(i, P)],
                                    kx[:, ds(2 * u, 2), ds(512 * ct + 256 * cc, 256)],
                                    start=(u == 0),
                                    stop=(u == 3),
                                    perf_mode=PM.DoubleRow,
                                )
                        st = stg.tile([P, 512], F8, tag="stg")
                        nc.vector.tensor_scalar(
                            st[:], ps[:], 0.25, bk_sb[:, i : i + 1], MULT, ADD
                        )
                        nc.sync.dma_start(khT_d.ap()[ts(i, P), ts(ct, 512)], st[:])

                # V projection (swapped operands, bf16): psum [sk-tile,
                # hk-chunk] -> copy into vaug (ones cols pre-set by memset)
                for t in range(16 if ('proj' in phases or 'v' in phases) else 0):
                    for c in range(2):
                        ps = pp.tile([P, 512], F32, tag="pp", name=f"psv{t}_{c}")
                        for u in range(8):
                            nc.tensor.matmul(
                                ps[:],
                                vx[:, u, ts(t, P)],
                                wv_sb[:, u, ds(512 * c, 512)],
                                start=(u == 0),
                                stop=False,
                            )
                        nc.tensor.matmul(
                            ps[:],
                            ones16[0:1, 0:P],
                            bvr_sb[:, ds(512 * c, 512)],
                            start=False,
                            stop=True,
                        )
                        nc.vector.tensor_copy(
                            vaug[:, t, ds(520 * c, 520)]
                            .rearrange("p (h x) -> p h x", x=65)[:, :, 0:64],
                            ps[:].rearrange("p (h x) -> p h x", x=64),
                        )

            # deferred big loads: mask (quartered) + Wo + bo ride the DMA
            # queues while the projections run
            for quar in range(4):
                nc.sync.dma_start(
                    msk_sb[:, ds(4 * quar, 4), :],
                    mskT.ap().rearrange("(t p) s -> p t s", p=P)[
                        :, ds(4 * quar, 4), :
                    ],
                )
            nc.sync.dma_start(bor_sb[:], bor.ap())
            for _j in range(8):
                nc.sync.dma_start(
                    wo_sb[:, _j],
                    wo16.ap().rearrange("(j p) m -> p j m", p=P)[:, _j],
                )

            # ---------------- attention ----------------
            if 'attn' not in phases:
                nc.compile()
                return nc
            with (
                tc.tile_pool(name="qkp", bufs=2) as qkp,
                tc.tile_pool(name="ep", bufs=4) as ep,
                tc.tile_pool(name="sps", bufs=2, space="PSUM") as sps,
                tc.tile_pool(name="avp", bufs=1, space="PSUM") as avp,
                tc.tile_pool(name="totp", bufs=2) as totp,
                tc.tile_pool(name="rbp", bufs=2) as rbp,
                tc.tile_pool(name="rbps", bufs=1, space="PSUM") as rbps,
            ):
                onesr = rbp.tile([1, 64], FR, tag="onesr")
                nc.vector.tensor_copy(onesr[:], ones16[0:1, 0:64])
                qk_tiles = {}
                def load_head(h):
                    qhs = qkp.tile([32, 2, SQ], F8, tag="qhs", name=f"qhs{h}")
                    khs = qkp.tile([32, 2, SK], F8, tag="khs", name=f"khs{h}")
                    nc.sync.dma_start(
                        qhs[:],
                        qhT_d.ap().rearrange(
                            "(h half p) s -> p h half s", half=2, p=32
                        )[:, h],
                    )
                    nc.sync.dma_start(
                        khs[:],
                        khT_d.ap().rearrange(
                            "(h half p) s -> p h half s", half=2, p=32
                        )[:, h],
                    )
                    qk_tiles[h] = (qhs, khs)

                load_head(0)
                load_head(1)
                pending = [None]
                for h in range(H):
                    qhs, khs = qk_tiles.pop(h)
                    if h + 2 < H:
                        load_head(h + 2)
                    pso = avp.tile([65, SQ], F32, tag="pso", name=f"pso{h}")
                    for t in range(16):
                        if t == 2 and pending[0] is not None:
                            pending[0]()
                            pending[0] = None
                        ps = sps.tile([P, SQ], F32, tag="sps", name=f"s{h}_{t}")
                        for c in range(4):
                            nc.tensor.matmul(
                                ps[:, ds(256 * c, 256)],
                                khs[:, :, ts(t, P)],
                                qhs[:, :, ds(256 * c, 256)],
                                start=True,
                                stop=True,
                                perf_mode=PM.DoubleRow,
                            )
                        e = ep.tile([P, SQ], BF, tag="e", name=f"e{h}_{t}")
                        nc.scalar.activation(e[:], ps[:], AF.Exp, scale=2.0 ** -9)
                        nc.vector.copy_predicated(e[:], msk_sb[:, t, :], ones16[:])
                        for c2 in range(2):
                            nc.tensor.matmul(
                                pso[:, ds(512 * c2, 512)],
                                vaug[:, t, ds(65 * h, 65)],
                                e[:, ds(512 * c2, 512)],
                                start=(t == 0),
                                stop=(t == 15),
                            )
                    def make_norm(h, pso):
                        def emit():
                            tot = totp.tile([65, SQ], F32, tag="tot", name=f"tot{h}")
                            nc.scalar.activation(tot[:], pso[:], AF.Copy, scale=1.0)
                            rcp = rbp.tile([1, SQ], FR, tag="rcp", name=f"rcp{h}")
                            with nc.allow_low_precision(reason="f32r recip, multiplicative use"):
                                nc.vector.reciprocal(rcp[:], tot[64:65, :])
                            rb = rbps.tile([64, SQ], F32, tag="rb", name=f"rb{h}")
                            for _cb in range(2):
                                nc.tensor.matmul(
                                    rb[:, ds(512 * _cb, 512)],
                                    onesr[:, 0:64],
                                    rcp[:, ds(512 * _cb, 512)],
                                    start=True,
                                    stop=True,
                                )
                            nc.vector.scalar_tensor_tensor(
                                ao_sb[ds(64 * (h % 2), 64), h // 2, :],
                                tot[0:64, :],
                                1.0,
                                rb[:],
                                MULT,
                                MULT,
                            )
                        return emit
                    pending[0] = make_norm(h, pso)
                pending[0]()

            # ---------------- output projection ----------------
            if 'oproj' not in phases:
                nc.compile()
                return nc
            with (
                tc.tile_pool(name="pp2", bufs=4, space="PSUM") as pp2,
                tc.tile_pool(name="ost", bufs=3) as ost,
            ):
                for s in range(8):
                    for c in range(2):
                        ps = pp2.tile([P, 512], F32, tag="op", name=f"po{s}_{c}")
                        for g in range(8):
                            nc.tensor.matmul(
                                ps[:],
                                ao_sb[:, g, ts(s, P)],
                                wo_sb[:, g, ds(512 * c, 512)],
                                start=(g == 0),
                                stop=False,
                            )
                        nc.tensor.matmul(
                            ps[:],
                            ones16[0:1, 0:P],
                            bor_sb[:, ds(512 * c, 512)],
                            start=False,
                            stop=True,
                        )
                        oo = ost.tile([P, 512], F32, tag="oo")
                        nc.scalar.activation(oo[:], ps[:], AF.Copy, scale=1.0)
                        nc.sync.dma_start(out.ap()[ts(s, P), ds(512 * c, 512)], oo[:])

    nc.compile()
    return nc


def make_host_inputs(q, k, v, mask, Wq, bq, Wk, bk, Wv, bv, Wo, bo):
    """Full inputs -> list of 8 per-core input dicts."""
    q = np.asarray(q, np.float32)
    k = np.asarray(k, np.float32)
    v = np.asarray(v, np.float32)
    mask = np.asarray(mask)

    def f8(a):
        return np.ascontiguousarray(a, dtype=np.float32).astype(NF8)

    def bf(a):
        return np.ascontiguousarray(a, dtype=np.float32).astype(NBF)

    shared = {
        "wq8": f8(32.0 * np.asarray(Wq, np.float32).transpose(1, 0, 2).reshape(D, HK)),
        "wk8": f8(32.0 * np.asarray(Wk, np.float32).transpose(1, 0, 2).reshape(D, HK)),
        "wv16": bf(np.asarray(Wv, np.float32).transpose(1, 0, 2).reshape(D, HK)),
        "wo16": bf(np.asarray(Wo, np.float32)),
        "bqc": np.ascontiguousarray(
            8.0 * np.asarray(bq, np.float32).reshape(HK).reshape(8, P).T
        ),
        "bkc": np.ascontiguousarray(
            8.0 * np.asarray(bk, np.float32).reshape(HK).reshape(8, P).T
        ),
        "bvr": bf(np.asarray(bv, np.float32).reshape(1, HK)),
        "bor": bf(np.asarray(bo, np.float32).reshape(1, D)),
    }

    in_maps = []
    for core in range(8):
        b, j = divmod(core, 2)
        qs = q[b, j * SQ : (j + 1) * SQ, :]
        ms = mask[b, j * SQ : (j + 1) * SQ, :]
        m = dict(shared)
        m["qT8"] = f8(qs.T)
        m["kT8"] = f8(k[b].T)
        m["vT16"] = bf(v[b].T)
        m["mskT"] = np.ascontiguousarray((~ms).T).astype(np.uint16)
        in_maps.append(m)
    return in_maps


def assemble_output(results):
    """8 per-core out [SQ, D] -> full [4, 2048, 1024]."""
    B, S = 4, 2048
    full = np.empty((B, S, D), np.float32)
    for core, res in enumerate(results):
        b, j = divmod(core, 2)
        full[b, j * SQ : (j + 1) * SQ, :] = res["out"]
    return full


class CompiledSpmd:
    def __init__(self, nc: bass.Bass, n_cores: int):
        bass2jax.install_neuronx_cc_hook()
        assert nc.dbg_addr is None, "build with debug=False"
        partition_name = (
            nc.partition_id_tensor.name if nc.partition_id_tensor else None
        )
        in_names, out_names, out_avals, zero_outs = [], [], [], []
        for alloc in nc.m.functions[0].allocations:
            if not isinstance(alloc, mybir.MemoryLocationSet):
                continue
            name = alloc.memorylocations[0].name
            if alloc.kind == "ExternalInput":
                if name != partition_name:
                    in_names.append(name)
            elif alloc.kind == "ExternalOutput":
                shape = tuple(alloc.tensor_shape)
                dtype = mybir.dt.np(alloc.dtype)
                out_names.append(name)
                out_avals.append(jax.core.ShapedArray(shape, dtype))
                zero_outs.append(np.zeros(shape, dtype))
        n_params = len(in_names)
        n_outs = len(out_avals)
        all_in_names = list(in_names) + list(out_names)
        if partition_name is not None:
            all_in_names.append(partition_name)

        def _body(*args):
            operands = list(args)
            if partition_name is not None:
                operands.append(bass2jax.partition_id_tensor())
            outs = bass2jax._bass_exec_p.bind(
                *operands,
                out_avals=tuple(out_avals),
                in_names=tuple(all_in_names),
                out_names=tuple(out_names),
                lowering_input_output_aliases=(),
                sim_require_finite=True,
                sim_require_nnan=True,
                nc=nc,
            )
            return tuple(outs)

        devices = jax.devices()[:n_cores]
        assert len(devices) == n_cores
        mesh = Mesh(np.asarray(devices), ("core",))
        self._mesh = mesh
        donate = tuple(range(n_params, n_params + n_outs))
        self._sharded = jax.jit(
            shard_map(
                _body,
                mesh=mesh,
                in_specs=(PartitionSpec("core"),) * (n_params + n_outs),
                out_specs=(PartitionSpec("core"),) * n_outs,
                check_rep=False,
            ),
            donate_argnums=donate,
            keep_unused=True,
        )
        self.in_names = in_names
        self.out_names = out_names
        self.out_avals = out_avals
        self.zero_outs = zero_outs
        self.n_cores = n_cores

    def _concat_inputs(self, in_maps):
        per_core = [[np.asarray(m[n]) for n in self.in_names] for m in in_maps]
        return [
            np.concatenate([per_core[c][i] for c in range(self.n_cores)], axis=0)
            for i in range(len(self.in_names))
        ]

    def run(self, in_maps, repeats: int = 1):
        """Returns (results_per_core, wall_times_s list of len repeats)."""
        from jax.sharding import NamedSharding

        mesh = self._mesh
        shard = NamedSharding(mesh, PartitionSpec("core"))
        concat_in = [
            jax.device_put(a, shard) for a in self._concat_inputs(in_maps)
        ]
        rep_zeros = [
            [
                jax.device_put(
                    np.zeros((self.n_cores * z.shape[0], *z.shape[1:]), z.dtype),
                    shard,
                )
                for z in self.zero_outs
            ]
            for _ in range(repeats)
        ]
        jax.block_until_ready(concat_in)
        jax.block_until_ready(rep_zeros)
        times = []
        out_arrs = None
        for r in range(repeats):
            t0 = time.perf_counter()
            out_arrs = self._sharded(*concat_in, *rep_zeros[r])
            jax.block_until_ready(out_arrs)
            times.append(time.perf_counter() - t0)
        results = [
            {
                name: np.asarray(out_arrs[i]).reshape(
                    self.n_cores, *self.out_avals[i].shape
                )[c]
                for i, name in enumerate(self.out_names)
            }
            for c in range(self.n_cores)
        ]
        return results, times


_COMPILED = None


def _get_compiled():
    global _COMPILED
    if _COMPILED is None:
        nc = build_mha()
        _COMPILED = CompiledSpmd(nc, 8)
    return _COMPILED


def kernel(**inputs) -> np.ndarray:
    comp = _get_compiled()
    in_maps = make_host_inputs(**inputs)
    results, _ = comp.run(in_maps, repeats=1)
    return assemble_output(results)


# revision 32
# speedup vs baseline: 1.0400x; 1.0046x over previous
"""Self-contained Trainium2 Bass kernel for nn_MultiHeadAttention_68367289417808.

kernel(**inputs) takes FULL unsharded inputs (as in reference.setup_inputs())
and returns the FULL [4, 2048, 1024] output.

Sharding: 8 cores = (batch 4) x (query-half 2); no collectives needed.

Per-core pipeline (mixed precision, tuned against the TRN2 cost model):
  - Q/K projections in fp8e4m3 DoubleRow (weights x32, outputs stored as
    8*qh in fp8), V projection in fp8e4m3 DoubleRow with vh stored bf16.
  - scores = kh^T qh per head via fp8 DoubleRow (dk=64 split into two
    32-row k-tiles living on the same 32 partitions).
  - exp on the Act engine straight out of PSUM into bf16 (scale 2^-9
    compensates the 8x8 operand scaling); masked positions are then
    overwritten with exactly 1.0 (= exp(-1e-6) to ulp) via one DVE
    copy_predicated against a ones tile, matching the reference's
    masked_fill(-1e-6) semantics with no correction terms.
  - AV + softmax denominator (ones column in vaug) in bf16 matmuls,
    normalize on DVE (reciprocal + scalar_tensor_tensor), output
    projection + bias in bf16 with the final copy on the Act engine.
"""
import time

import jax
import ml_dtypes
import numpy as np
from jax.experimental.shard_map import shard_map
from jax.sharding import Mesh, PartitionSpec

import concourse.bass as bass
import concourse.bacc as bacc
import concourse.mybir as mybir
import concourse.tile as tile
from concourse import bass2jax
from concourse.bass import ts, ds

F32 = mybir.dt.float32
F8 = mybir.dt.float8e4
BF = mybir.dt.bfloat16
U16 = mybir.dt.uint16
U8 = mybir.dt.uint8
FR = mybir.dt.float32r
AF = mybir.ActivationFunctionType
PM = mybir.MatmulPerfMode
MULT = mybir.AluOpType.mult
ADD = mybir.AluOpType.add

NF8 = ml_dtypes.float8_e4m3
NBF = ml_dtypes.bfloat16

P = 128
SQ = 1024
SK = 2048
D = 1024
H = 16
DK = 64
HK = 1024


def build_mha(phases=('proj', 'attn', 'oproj')):
    nc = bacc.Bacc("TRN2", target_bir_lowering=False)

    qT8 = nc.dram_tensor("qT8", [D, SQ], F8, kind="ExternalInput")
    kT8 = nc.dram_tensor("kT8", [D, SK], F8, kind="ExternalInput")
    vT16 = nc.dram_tensor("vT16", [D, SK], BF, kind="ExternalInput")
    mskT = nc.dram_tensor("mskT", [SK, SQ], U8, kind="ExternalInput")
    wq8 = nc.dram_tensor("wq8", [D, HK], F8, kind="ExternalInput")
    wk8 = nc.dram_tensor("wk8", [D, HK], F8, kind="ExternalInput")
    wv16 = nc.dram_tensor("wv16", [D, HK], BF, kind="ExternalInput")
    wo16 = nc.dram_tensor("wo16", [HK, D], BF, kind="ExternalInput")
    bqc = nc.dram_tensor("bqc", [P, 8], F32, kind="ExternalInput")
    bkc = nc.dram_tensor("bkc", [P, 8], F32, kind="ExternalInput")
    bvr = nc.dram_tensor("bvr", [1, HK], BF, kind="ExternalInput")
    bor = nc.dram_tensor("bor", [1, D], BF, kind="ExternalInput")
    out = nc.dram_tensor("out", [SQ, D], F32, kind="ExternalOutput")

    qhT_d = nc.dram_tensor("qhT_scr", [HK, SQ], F8)
    khT_d = nc.dram_tensor("khT_scr", [HK, SK], F8)
    rden_d = nc.dram_tensor("rden_scr", [H, SQ], F32)

    with tile.TileContext(nc) as tc:
        with tc.tile_pool(name="persist", bufs=1) as pers:
            msk_sb = pers.tile([P, 16, SQ], U8, tag="msk")
            vaug = pers.tile([P, 16, 65 * H], BF, tag="vaug")
            ao_sb = pers.tile([P, 8, SQ], BF, tag="ao")
            wo_sb = pers.tile([P, 8, D], BF, tag="wo")
            ones16 = pers.tile([P, SQ], BF, tag="ones16")
            bor_sb = pers.tile([1, D], BF, tag="bor")

            nc.vector.memset(ones16[:], 1.0)
            nc.gpsimd.memset(vaug[:].rearrange("p t c -> p (t c)"), 1.0)

            # ---------------- Q/K/V projections ----------------
            with (
                tc.tile_pool(name="wp", bufs=1) as wp,
                tc.tile_pool(name="xp", bufs=1) as xp,
                tc.tile_pool(name="pp", bufs=4, space="PSUM") as pp,
                tc.tile_pool(name="stg", bufs=3) as stg,
            ):
                wq_sb = wp.tile([P, 8, HK], F8, tag="wq")
                wk_sb = wp.tile([P, 8, HK], F8, tag="wk")
                wv_sb = wp.tile([P, 8, HK], BF, tag="wv")
                bvr_sb = wp.tile([1, HK], BF, tag="bvr")
                bq_sb = wp.tile([P, 8], F32, tag="bq")
                bk_sb = wp.tile([P, 8], F32, tag="bk")
                qx = xp.tile([P, 8, SQ], F8, tag="qx")
                kx = xp.tile([P, 8, SK], F8, tag="kx")
                vx = xp.tile([P, 8, SK], BF, tag="vx")
                for _j in range(8):
                    nc.sync.dma_start(
                        wq_sb[:, _j],
                        wq8.ap().rearrange("(j p) m -> p j m", p=P)[:, _j],
                    )
                    nc.sync.dma_start(
                        wk_sb[:, _j],
                        wk8.ap().rearrange("(j p) m -> p j m", p=P)[:, _j],
                    )
                    nc.sync.dma_start(
                        wv_sb[:, _j],
                        wv16.ap().rearrange("(j p) m -> p j m", p=P)[:, _j],
                    )
                    nc.sync.dma_start(
                        qx[:, _j],
                        qT8.ap().rearrange("(j p) s -> p j s", p=P)[:, _j],
                    )
                    nc.sync.dma_start(
                        kx[:, _j],
                        kT8.ap().rearrange("(j p) s -> p j s", p=P)[:, _j],
                    )
                    nc.sync.dma_start(
                        vx[:, _j],
                        vT16.ap().rearrange("(j p) s -> p j s", p=P)[:, _j],
                    )
                nc.sync.dma_start(bvr_sb[:], bvr.ap())
                nc.sync.dma_start(bq_sb[:], bqc.ap())
                nc.sync.dma_start(bk_sb[:], bkc.ap())

                # Q projection -> qhT_d (fp8, value 8*qh)
                for i in range(8 if ('proj' in phases or 'q' in phases) else 0):
                    for ct in range(SQ // 512):
                        ps = pp.tile([P, 512], F32, tag="pp", name=f"psq{i}_{ct}")
                        for cc in range(2):
                            for u in range(4):
                                nc.tensor.matmul(
                                    ps[:, ds(256 * cc, 256)],
                                    wq_sb[:, ds(2 * u, 2), ts(i, P)],
                                    qx[:, ds(2 * u, 2), ds(512 * ct + 256 * cc, 256)],
                                    start=(u == 0),
                                    stop=(u == 3),
                                    perf_mode=PM.DoubleRow,
                                )
                        st = stg.tile([P, 512], F8, tag="stg")
                        nc.vector.tensor_scalar(
                            st[:], ps[:], 0.25, bq_sb[:, i : i + 1], MULT, ADD
                        )
                        nc.sync.dma_start(qhT_d.ap()[ts(i, P), ts(ct, 512)], st[:])

                # K projection -> khT_d (fp8, value 8*kh)
                for i in range(8 if ('proj' in phases or 'k' in phases) else 0):
                    for ct in range(SK // 512):
                        ps = pp.tile([P, 512], F32, tag="pp", name=f"psk{i}_{ct}")
                        for cc in range(2):
                            for u in range(4):
                                nc.tensor.matmul(
                                    ps[:, ds(256 * cc, 256)],
                                    wk_sb[:, ds(2 * u, 2), ts(i, P)],
                                    kx[:, ds(2 * u, 2), ds(512 * ct + 256 * cc, 256)],
                                    start=(u == 0),
                                    stop=(u == 3),
                                    perf_mode=PM.DoubleRow,
                                )
                        st = stg.tile([P, 512], F8, tag="stg")
                        nc.vector.tensor_scalar(
                            st[:], ps[:], 0.25, bk_sb[:, i : i + 1], MULT, ADD
                        )
                        nc.sync.dma_start(khT_d.ap()[ts(i, P), ts(ct, 512)], st[:])

                # V projection (swapped operands, bf16): psum [sk-tile,
                # hk-chunk] -> copy into vaug (ones cols pre-set by memset)
                for t in range(16 if ('proj' in phases or 'v' in phases) else 0):
                    for c in range(2):
                        ps = pp.tile([P, 512], F32, tag="pp", name=f"psv{t}_{c}")
                        for u in range(8):
                            nc.tensor.matmul(
                                ps[:],
                                vx[:, u, ts(t, P)],
                                wv_sb[:, u, ds(512 * c, 512)],
                                start=(u == 0),
                                stop=False,
                            )
                        nc.tensor.matmul(
                            ps[:],
                            ones16[0:1, 0:P],
                            bvr_sb[:, ds(512 * c, 512)],
                            start=False,
                            stop=True,
                        )
                        nc.vector.tensor_copy(
                            vaug[:, t, ds(520 * c, 520)]
                            .rearrange("p (h x) -> p h x", x=65)[:, :, 0:64],
                            ps[:].rearrange("p (h x) -> p h x", x=64),
                        )

            # deferred big loads: mask (quartered) + Wo + bo ride the DMA
            # queues while the projections run
            for quar in range(4):
                nc.sync.dma_start(
                    msk_sb[:, ds(4 * quar, 4), :],
                    mskT.ap().rearrange("(t p) s -> p t s", p=P)[
                        :, ds(4 * quar, 4), :
                    ],
                )
            nc.sync.dma_start(bor_sb[:], bor.ap())
            for _j in range(8):
                nc.sync.dma_start(
                    wo_sb[:, _j],
                    wo16.ap().rearrange("(j p) m -> p j m", p=P)[:, _j],
                )

            # ---------------- attention ----------------
            if 'attn' not in phases:
                nc.compile()
                return nc
            with (
                tc.tile_pool(name="qkp", bufs=3) as qkp,
                tc.tile_pool(name="ep", bufs=4) as ep,
                tc.tile_pool(name="sps", bufs=2, space="PSUM") as sps,
                tc.tile_pool(name="avp", bufs=1, space="PSUM") as avp,
                tc.tile_pool(name="totp", bufs=2) as totp,
                tc.tile_pool(name="rbp", bufs=2) as rbp,
                tc.tile_pool(name="rbps", bufs=1, space="PSUM") as rbps,
            ):
                onesr = rbp.tile([1, 64], FR, tag="onesr")
                nc.vector.tensor_copy(onesr[:], ones16[0:1, 0:64])
                qk_tiles = {}
                def load_head(h):
                    qhs = qkp.tile([32, 2, SQ], F8, tag="qhs", name=f"qhs{h}")
                    khs = qkp.tile([32, 2, SK], F8, tag="khs", name=f"khs{h}")
                    nc.sync.dma_start(
                        qhs[:],
                        qhT_d.ap().rearrange(
                            "(h half p) s -> p h half s", half=2, p=32
                        )[:, h],
                    )
                    nc.sync.dma_start(
                        khs[:],
                        khT_d.ap().rearrange(
                            "(h half p) s -> p h half s", half=2, p=32
                        )[:, h],
                    )
                    qk_tiles[h] = (qhs, khs)

                load_head(0)
                load_head(1)
                pending = [None]
                for h in range(H):
                    qhs, khs = qk_tiles.pop(h)
                    if h + 2 < H:
                        load_head(h + 2)
                    pso = avp.tile([65, SQ], F32, tag="pso", name=f"pso{h}")
                    for t in range(16):
                        if t == 2 and pending[0] is not None:
                            pending[0]()
                            pending[0] = None
                        ps = sps.tile([P, SQ], F32, tag="sps", name=f"s{h}_{t}")
                        for c in range(4):
                            nc.tensor.matmul(
                                ps[:, ds(256 * c, 256)],
                                khs[:, :, ts(t, P)],
                                qhs[:, :, ds(256 * c, 256)],
                                start=True,
                                stop=True,
                                perf_mode=PM.DoubleRow,
                            )
                        e = ep.tile([P, SQ], BF, tag="e", name=f"e{h}_{t}")
                        nc.scalar.activation(e[:], ps[:], AF.Exp, scale=2.0 ** -9)
                        nc.vector.copy_predicated(e[:], msk_sb[:, t, :], ones16[:])
                        for c2 in range(2):
                            nc.tensor.matmul(
                                pso[:, ds(512 * c2, 512)],
                                vaug[:, t, ds(65 * h, 65)],
                                e[:, ds(512 * c2, 512)],
                                start=(t == 0),
                                stop=(t == 15),
                            )
                    def make_norm(h, pso):
                        def emit():
                            tot = totp.tile([65, SQ], F32, tag="tot", name=f"tot{h}")
                            nc.scalar.activation(tot[:], pso[:], AF.Copy, scale=1.0)
                            rcp = rbp.tile([1, SQ], FR, tag="rcp", name=f"rcp{h}")
                            with nc.allow_low_precision(reason="f32r recip, multiplicative use"):
                                nc.vector.reciprocal(rcp[:], tot[64:65, :])
                            rb = rbps.tile([64, SQ], F32, tag="rb", name=f"rb{h}")
                            for _cb in range(2):
                                nc.tensor.matmul(
                                    rb[:, ds(512 * _cb, 512)],
                                    onesr[:, 0:64],
                                    rcp[:, ds(512 * _cb, 512)],
                                    start=True,
                                    stop=True,
                                )
                            nc.vector.scalar_tensor_tensor(
                                ao_sb[ds(64 * (h % 2), 64), h // 2, :],
                                tot[0:64, :],
                                1.0,
                                rb[:],
                                MULT,
                                MULT,
                            )
                        return emit
                    pending[0] = make_norm(h, pso)
                pending[0]()

            # ---------------- output projection ----------------
            if 'oproj' not in phases:
                nc.compile()
                return nc
            with (
                tc.tile_pool(name="pp2", bufs=4, space="PSUM") as pp2,
                tc.tile_pool(name="ost", bufs=3) as ost,
            ):
                for s in range(8):
                    for c in range(2):
                        ps = pp2.tile([P, 512], F32, tag="op", name=f"po{s}_{c}")
                        for g in range(8):
                            nc.tensor.matmul(
                                ps[:],
                                ao_sb[:, g, ts(s, P)],
                                wo_sb[:, g, ds(512 * c, 512)],
                                start=(g == 0),
                                stop=False,
                            )
                        nc.tensor.matmul(
                            ps[:],
                            ones16[0:1, 0:P],
                            bor_sb[:, ds(512 * c, 512)],
                            start=False,
                            stop=True,
                        )
                        oo = ost.tile([P, 512], F32, tag="oo")
                        nc.scalar.activation(oo[:], ps[:], AF.Copy, scale=1.0)
                        nc.sync.dma_start(out.ap()[ts(s, P), ds(512 * c, 512)], oo[:])

    nc.compile()
    return nc


def make_host_inputs(q, k, v, mask, Wq, bq, Wk, bk, Wv, bv, Wo, bo):
    """Full inputs -> list of 8 per-core input dicts."""
    q = np.asarray(q, np.float32)
    k = np.asarray(k, np.float32)
    v = np.asarray(v, np.float32)
    mask = np.asarray(mask)

    def f8(a):
        return np.ascontiguousarray(a, dtype=np.float32).astype(NF8)

    def bf(a):
        return np.ascontiguousarray(a, dtype=np.float32).astype(NBF)

    shared = {
        "wq8": f8(32.0 * np.asarray(Wq, np.float32).transpose(1, 0, 2).reshape(D, HK)),
        "wk8": f8(32.0 * np.asarray(Wk, np.float32).transpose(1, 0, 2).reshape(D, HK)),
        "wv16": bf(np.asarray(Wv, np.float32).transpose(1, 0, 2).reshape(D, HK)),
        "wo16": bf(np.asarray(Wo, np.float32)),
        "bqc": np.ascontiguousarray(
            8.0 * np.asarray(bq, np.float32).reshape(HK).reshape(8, P).T
        ),
        "bkc": np.ascontiguousarray(
            8.0 * np.asarray(bk, np.float32).reshape(HK).reshape(8, P).T
        ),
        "bvr": bf(np.asarray(bv, np.float32).reshape(1, HK)),
        "bor": bf(np.asarray(bo, np.float32).reshape(1, D)),
    }

    in_maps = []
    for core in range(8):
        b, j = divmod(core, 2)
        qs = q[b, j * SQ : (j + 1) * SQ, :]
        ms = mask[b, j * SQ : (j + 1) * SQ, :]
        m = dict(shared)
        m["qT8"] = f8(qs.T)
        m["kT8"] = f8(k[b].T)
        m["vT16"] = bf(v[b].T)
        m["mskT"] = np.ascontiguousarray((~ms).T).astype(np.uint8)
        in_maps.append(m)
    return in_maps


def assemble_output(results):
    """8 per-core out [SQ, D] -> full [4, 2048, 1024]."""
    B, S = 4, 2048
    full = np.empty((B, S, D), np.float32)
    for core, res in enumerate(results):
        b, j = divmod(core, 2)
        full[b, j * SQ : (j + 1) * SQ, :] = res["out"]
    return full


class CompiledSpmd:
    def __init__(self, nc: bass.Bass, n_cores: int):
        bass2jax.install_neuronx_cc_hook()
        assert nc.dbg_addr is None, "build with debug=False"
        partition_name = (
            nc.partition_id_tensor.name if nc.partition_id_tensor else None
        )
        in_names, out_names, out_avals, zero_outs = [], [], [], []
        for alloc in nc.m.functions[0].allocations:
            if not isinstance(alloc, mybir.MemoryLocationSet):
                continue
            name = alloc.memorylocations[0].name
            if alloc.kind == "ExternalInput":
                if name != partition_name:
                    in_names.append(name)
            elif alloc.kind == "ExternalOutput":
                shape = tuple(alloc.tensor_shape)
                dtype = mybir.dt.np(alloc.dtype)
                out_names.append(name)
                out_avals.append(jax.core.ShapedArray(shape, dtype))
                zero_outs.append(np.zeros(shape, dtype))
        n_params = len(in_names)
        n_outs = len(out_avals)
        all_in_names = list(in_names) + list(out_names)
        if partition_name is not None:
            all_in_names.append(partition_name)

        def _body(*args):
            operands = list(args)
            if partition_name is not None:
                operands.append(bass2jax.partition_id_tensor())
            outs = bass2jax._bass_exec_p.bind(
                *operands,
                out_avals=tuple(out_avals),
                in_names=tuple(all_in_names),
                out_names=tuple(out_names),
                lowering_input_output_aliases=(),
                sim_require_finite=True,
                sim_require_nnan=True,
                nc=nc,
            )
            return tuple(outs)

        devices = jax.devices()[:n_cores]
        assert len(devices) == n_cores
        mesh = Mesh(np.asarray(devices), ("core",))
        self._mesh = mesh
        donate = tuple(range(n_params, n_params + n_outs))
        self._sharded = jax.jit(
            shard_map(
                _body,
                mesh=mesh,
                in_specs=(PartitionSpec("core"),) * (n_params + n_outs),
                out_specs=(PartitionSpec("core"),) * n_outs,
                check_rep=False,
            ),
            donate_argnums=donate,
            keep_unused=True,
        )
        self.in_names = in_names
        self.out_names = out_names
        self.out_avals = out_avals
        self.zero_outs = zero_outs
        self.n_cores = n_cores

    def _concat_inputs(self, in_maps):
        per_core = [[np.asarray(m[n]) for n in self.in_names] for m in in_maps]
        return [
            np.concatenate([per_core[c][i] for c in range(self.n_cores)], axis=0)
            for i in range(len(self.in_names))
        ]

    def run(self, in_maps, repeats: int = 1):
        """Returns (results_per_core, wall_times_s list of len repeats)."""
        from jax.sharding import NamedSharding

        mesh = self._mesh
        shard = NamedSharding(mesh, PartitionSpec("core"))
        concat_in = [
            jax.device_put(a, shard) for a in self._concat_inputs(in_maps)
        ]
        rep_zeros = [
            [
                jax.device_put(
                    np.zeros((self.n_cores * z.shape[0], *z.shape[1:]), z.dtype),
                    shard,
                )
                for z in self.zero_outs
            ]
            for _ in range(repeats)
        ]
        jax.block_until_ready(concat_in)
        jax.block_until_ready(rep_zeros)
        times = []
        out_arrs = None
        for r in range(repeats):
            t0 = time.perf_counter()
            out_arrs = self._sharded(*concat_in, *rep_zeros[r])
            jax.block_until_ready(out_arrs)
            times.append(time.perf_counter() - t0)
        results = [
            {
                name: np.asarray(out_arrs[i]).reshape(
                    self.n_cores, *self.out_avals[i].shape
                )[c]
                for i, name in enumerate(self.out_names)
            }
            for c in range(self.n_cores)
        ]
        return results, times


_COMPILED = None


def _get_compiled():
    global _COMPILED
    if _COMPILED is None:
        nc = build_mha()
        _COMPILED = CompiledSpmd(nc, 8)
    return _COMPILED


def kernel(**inputs) -> np.ndarray:
    comp = _get_compiled()
    in_maps = make_host_inputs(**inputs)
    results, _ = comp.run(in_maps, repeats=1)
    return assemble_output(results)
